# revision 2
# baseline (speedup 1.0000x reference)
"""Multi-relational GCN (4 layers) on 8 TRN2 cores — ap_gather pipeline.

Layout is transposed throughout: features on partitions, nodes/edge-slots on
the free dimension. Per-edge source rows are fetched with gpsimd.ap_gather
(SBUF free-dim gather, no DMA descriptors) from an SBUF-resident window of
the source table. Relations are processed in PAIRS (2i, 2i+1): partitions
0-63 carry relation 2i, 64-127 carry 2i+1, so all 8 GPSIMD cores are busy
and one stacked-weight matmul [W_2i; W_2i+1]^T @ tmp sums both relations.

Per (pair, window) subset: ELL (degree-bucketed, K slots per node,
contiguous) gather -> multiply by edge vals (broadcast-DMAed to partitions)
-> tensor_reduce over K -> per-window token table -> merge ap_gather
(token -> node) -> matmul accumulate into h_pre. Layer 0 gathers from
pre-transformed XW1_r tables and merges with a stacked-identity matmul.

h tables are exchanged via AllGather of [64, NPAD] shards into
[8, 64, NPAD] DRAM tables; windows (12544 nodes = 2 ranks) are DMAed into
SBUF per (pair, window).
"""
import numpy as np

import concourse.bacc as bacc
import concourse.mybir as mybir
import concourse.tile as tile
from concourse.bass_utils import run_bass_kernel_spmd
from concourse.masks import make_identity

N = 50000
NC = 8
NP = N // NC              # 6250 nodes per core per type
NPAD = 6272               # 49*128, padded per-core node count
NPALL = NC * NPAD         # 50176
NWIN = 4
WIN = NPALL // NWIN       # 12544 = 2 ranks per window
P = 128
D = 64
F_IN = 128
RELS = 4
BUCKETS = (1, 2, 3, 4, 5, 6, 8, 10, 12, 16, 20, 24, 32, 48, 64, 96, 128)
CH_SL = 2048              # gather chunk slot budget
MCH = 512                 # merge / matmul node chunk

F32 = mybir.dt.float32
I16 = mybir.dt.int16


# --------------------------------------------------------------------------
# host-side structure builder
# --------------------------------------------------------------------------

def _wrap16(a):
    n = a.shape[0]
    assert n % 16 == 0
    return a.reshape(n // 16, 16).T


def _build_structure(rows, cols, vals):
    """ELL structure for one adjacency, paired relations, 4 windows.

    Returns (consts, per_core):
      consts[(p, w)] = dict(groups, slot_base, tok_base, n_slots, n_tok,
                            zero_tok, chunks)  — identical across cores/halves
      per_core[c][(p, w)] = dict(gidx [2, n_slots] i16, gval [2, n_slots] f32,
                                 midx [2, NPAD] i16)
    """
    rows = np.asarray(rows).astype(np.int64)
    cols = np.asarray(cols).astype(np.int64)
    vals = np.asarray(vals).astype(np.float32)

    work = {}
    gc_all = {}
    for r in range(RELS):
        p, h = r // 2, r % 2
        rr, cc, vv = rows[r], cols[r], vals[r]
        core = rr // NP
        srow = (cc // NP) * NPAD + (cc % NP)
        win = srow // WIN
        for c in range(NC):
            mc = core == c
            for w in range(NWIN):
                m = mc & (win == w)
                dst = (rr[m] - c * NP).astype(np.int64)
                src = (srow[m] - w * WIN).astype(np.int32)
                val = vv[m]
                order = np.argsort(dst, kind="stable")
                dst, src, val = dst[order], src[order], val[order]
                counts = np.bincount(dst, minlength=NP)
                nodes = np.nonzero(counts)[0]
                degs = counts[nodes]
                kbi = np.searchsorted(np.asarray(BUCKETS), degs)
                gc = np.bincount(kbi, minlength=len(BUCKETS))
                work[(p, w, c, h)] = (dst, src, val, counts, nodes, kbi)
                gc_all.setdefault((p, w), []).append(gc)

    consts = {}
    from math import gcd
    for (p, w), gcs in gc_all.items():
        gmax = np.max(np.stack(gcs), axis=0)
        groups, slot_base, tok_base = [], [], []
        s_off = t_off = 0
        for bi, K in enumerate(BUCKETS):
            if gmax[bi] == 0:
                groups.append(None)
                slot_base.append(-1)
                tok_base.append(-1)
                continue
            # ap_gather ucode mishandles idx slices starting off a 16-byte
            # (128-idx) boundary: round each group's node count so both the
            # group's slot span and every chunk boundary are 128-slot aligned
            per = 128 // gcd(K, 128)
            gpad = int(-(-max(gmax[bi], 16) // per) * per)
            groups.append((K, gpad))
            slot_base.append(s_off)
            tok_base.append(t_off)
            s_off += gpad * K
            t_off += gpad
        # dummy all-zero group: guarantees a zero token for absent nodes
        zero_tok = t_off
        groups.append((1, 128))
        slot_base.append(s_off)
        tok_base.append(t_off)
        s_off += 128
        t_off += 128
        # chunk plan: (s0, ni, t0, cb, K); starts stay 128-slot aligned
        chunks = []
        for bi, grp in enumerate(groups):
            if grp is None:
                continue
            K, gpad = grp
            per = 128 // gcd(K, 128)
            cb_full = min(gpad, max(per, (CH_SL // K) // per * per))
            g0 = 0
            while g0 < gpad:
                cb = min(cb_full, gpad - g0)
                chunks.append((slot_base[bi] + g0 * K, cb * K,
                               tok_base[bi] + g0, cb, K))
                g0 += cb
        consts[(p, w)] = dict(groups=groups, slot_base=slot_base,
                              tok_base=tok_base, n_slots=s_off, n_tok=t_off,
                              zero_tok=zero_tok, chunks=chunks)

    per_core = [dict() for _ in range(NC)]
    for (p, w, c, h), (dst, src, val, counts, nodes, kbi) in work.items():
        cst = consts[(p, w)]
        if (p, w) not in per_core[c]:
            per_core[c][(p, w)] = dict(
                gidx=np.zeros((2, cst["n_slots"]), np.int16),
                gval=np.zeros((2, cst["n_slots"]), np.float32),
                midx=np.full((2, NPAD), cst["zero_tok"], np.int16))
        d = per_core[c][(p, w)]
        m_of_node = np.zeros(NP, np.int64)
        b_of_node = np.zeros(NP, np.int64)
        for bi in range(len(BUCKETS)):
            if cst["groups"][bi] is None:
                continue
            gn = nodes[kbi == bi]
            m_of_node[gn] = np.arange(len(gn))
            b_of_node[gn] = bi
            d["midx"][h, gn] = cst["tok_base"][bi] + np.arange(len(gn))
        starts = np.zeros(NP + 1, np.int64)
        np.cumsum(counts, out=starts[1:])
        k_e = np.arange(len(dst)) - starts[dst]
        b_e = b_of_node[dst]
        K_e = np.asarray(BUCKETS)[b_e]
        pos = (np.asarray(cst["slot_base"])[b_e]
               + m_of_node[dst] * K_e + k_e)
        d["gidx"][h, pos] = src
        d["gval"][h, pos] = val
    return consts, per_core


def _concat_structures(structs):
    """-> (offsets[(a,p,w)] = (slot_off, subset_idx), s_tot,
           gidx_t, gval_t, midx_t per core)."""
    offsets = {}
    s_off = 0
    si = 0
    for a, (consts, _) in enumerate(structs):
        for p in range(2):
            for w in range(NWIN):
                offsets[(a, p, w)] = (s_off, si)
                s_off += consts[(p, w)]["n_slots"]
                si += 1
    n_sub = si
    gidx_t, gval_t, midx_t = [], [], []
    for c in range(NC):
        gi = np.zeros((2, s_off), np.int16)
        gv = np.zeros((2, s_off), np.float32)
        mi = np.zeros((2, n_sub * NPAD), np.int16)
        for a, (consts, per_core) in enumerate(structs):
            for p in range(2):
                for w in range(NWIN):
                    so, sidx = offsets[(a, p, w)]
                    d = per_core[c][(p, w)]
                    ns = d["gidx"].shape[1]
                    gi[:, so:so + ns] = d["gidx"]
                    gv[:, so:so + ns] = d["gval"]
                    mi[:, sidx * NPAD:(sidx + 1) * NPAD] = d["midx"]
        gidx_t.append(np.concatenate([_wrap16(gi[0]), _wrap16(gi[1])], 0).copy())
        gval_t.append(gv)
        midx_t.append(np.concatenate([_wrap16(mi[0]), _wrap16(mi[1])], 0).copy())
    return offsets, s_off, n_sub, gidx_t, gval_t, midx_t


# --------------------------------------------------------------------------
# device program
# --------------------------------------------------------------------------

_CONSTS = None
DEBUG = False


def build_program():
    structs_consts, offsets, s_tot, n_sub = _CONSTS
    nc = bacc.Bacc("TRN2", target_bir_lowering=False, debug=False,
                   num_devices=NC)
    t_feat = nc.dram_tensor("feat", [2, F_IN, NPAD], F32, kind="ExternalInput")
    t_w = [nc.dram_tensor(f"W{l + 1}", [RELS, F_IN if l == 0 else D, D], F32,
                          kind="ExternalInput") for l in range(4)]
    t_gidx = nc.dram_tensor("gidx", [32, s_tot // 16], I16, kind="ExternalInput")
    t_gval = nc.dram_tensor("gval", [2, s_tot], F32, kind="ExternalInput")
    t_midx = nc.dram_tensor("midx", [32, n_sub * NPAD // 16], I16,
                            kind="ExternalInput")
    t_out = nc.dram_tensor("out", [2, D, NPAD], F32, kind="ExternalOutput")
    if DEBUG:
        t_dxw = nc.dram_tensor("dxw", [RELS, D, NPAD], F32,
                               kind="ExternalOutput")
        t_dh0 = nc.dram_tensor("dh0", [2, D, NPAD], F32, kind="ExternalOutput")
        t_drt = nc.dram_tensor("drt", [P, 8192], F32, kind="ExternalOutput")

    with tile.TileContext(nc, num_cores=NC) as tc:
        with tc.tile_pool(name="cpool", bufs=1) as cpool, \
             tc.tile_pool(name="tpool", bufs=1) as tpool, \
             tc.tile_pool(name="hpool", bufs=1) as hpool, \
             tc.tile_pool(name="rpool", bufs=1) as rpool, \
             tc.tile_pool(name="gpool", bufs=2) as gpool, \
             tc.tile_pool(name="spool", bufs=2) as spool, \
             tc.tile_pool(name="psum", bufs=4, space="PSUM") as psum, \
             tc.tile_pool(name="dram", bufs=1, space="DRAM") as dram:

            # stacked identity [I64; I64] for layer-0 pair merges
            ident = cpool.tile([P, P], F32, tag="ident")
            make_identity(nc, ident[:])
            ident2 = cpool.tile([P, D], F32, tag="ident2")
            nc.scalar.copy(out=ident2[0:D, :], in_=ident[0:D, 0:D])
            nc.scalar.copy(out=ident2[D:P, :], in_=ident[D:P, D:P])

            # weights: W1 per relation [128, 64]; W2..4 stacked per pair
            w1t = []
            for r in range(RELS):
                wt = cpool.tile([F_IN, D], F32, tag=f"w1_{r}")
                nc.sync.dma_start(out=wt[:], in_=t_w[0][r])
                w1t.append(wt)
            wst = {}
            for l in range(1, 4):
                for p in range(2):
                    wt = cpool.tile([P, D], F32, tag=f"w{l}_{p}")
                    nc.sync.dma_start(out=wt[0:D, :], in_=t_w[l][2 * p])
                    nc.sync.dma_start(out=wt[D:P, :], in_=t_w[l][2 * p + 1])
                    wst[(l, p)] = wt

            # replicate wrapped idx tensors to 128 partitions in DRAM
            gidx_rep = dram.tile([P, s_tot // 16], I16, tag="gidx_rep")
            midx_rep = dram.tile([P, n_sub * NPAD // 16], I16, tag="midx_rep")
            for h in range(2):
                for g in range(4):
                    pr = 64 * h + 16 * g
                    nc.sync.dma_start(out=gidx_rep[pr:pr + 16, :],
                                      in_=t_gidx[16 * h:16 * h + 16, :])
                    nc.sync.dma_start(out=midx_rep[pr:pr + 16, :],
                                      in_=t_midx[16 * h:16 * h + 16, :])

            # DRAM tables
            xw1_full = [dram.tile([NC, D, NPAD], F32, tag=f"xw1f{r}",
                                  name=f"xw1f{r}", addr_space="Shared")
                        for r in range(RELS)]
            xw1_bounce = [dram.tile([D, NPAD], F32, tag=f"xw1b{r}",
                                    name=f"xw1b{r}") for r in range(RELS)]
            h_full = {}
            h_bounce = {}
            for l in range(3):
                for j in range(2):
                    h_full[(l, j)] = dram.tile([NC, D, NPAD], F32,
                                               tag=f"hf{l}_{j}",
                                               name=f"hf{l}_{j}",
                                               addr_space="Shared")
                    h_bounce[(l, j)] = dram.tile([D, NPAD], F32,
                                                 tag=f"hb{l}_{j}",
                                                 name=f"hb{l}_{j}")

            mm_chunks = [(c * MCH, min(MCH, NPAD - c * MCH))
                         for c in range(-(-NPAD // MCH))]

            # ---- stage A: XW1_r = (feat_j @ W1_r)^T, AllGather ----
            for j in range(2):
                for n0, nn in mm_chunks:
                    fch = spool.tile([F_IN, MCH], F32, tag="fch")
                    nc.sync.dma_start(out=fch[:, :nn],
                                      in_=t_feat[j, :, n0:n0 + nn])
                    for r in (j, j + 2):
                        ps = psum.tile([D, MCH], F32, tag="ps")
                        nc.tensor.matmul(out=ps[:, :nn], lhsT=w1t[r][:],
                                         rhs=fch[:, :nn], start=True, stop=True)
                        sb = spool.tile([D, MCH], F32, tag="sb")
                        nc.scalar.copy(out=sb[:, :nn], in_=ps[:, :nn])
                        nc.sync.dma_start(out=xw1_bounce[r][:, n0:n0 + nn],
                                          in_=sb[:, :nn])
            for r in range(RELS):
                nc.gpsimd.collective_compute(
                    "AllGather", mybir.AluOpType.bypass,
                    replica_groups=[list(range(NC))],
                    ins=[xw1_bounce[r].opt()],
                    outs=[xw1_full[r].opt()])
            if DEBUG:
                for r in range(RELS):
                    nc.sync.dma_start(out=t_dxw[r], in_=xw1_bounce[r][:])

            # ---- stage B: 4 GCN layers ----
            for l in range(4):
                a = 0 if l < 2 else 1
                consts = structs_consts[a]
                hpre = []
                for p in range(2):
                    ht = hpool.tile([D, NPAD], F32, tag=f"hpre{p}")
                    nc.vector.memset(ht[:], 0.0)
                    hpre.append(ht)
                for w in range(NWIN):
                    for p in range(2):
                        # table tile: halves = the pair's two source tables
                        if l == 0 or p == 0:
                            tt = tpool.tile([P, WIN], F32, tag="tt")
                            if l == 0:
                                srcs = (xw1_full[2 * p], xw1_full[2 * p + 1])
                            else:
                                srcs = (h_full[(l - 1, 0)], h_full[(l - 1, 1)])
                            for h in range(2):
                                nc.sync.dma_start(
                                    out=tt[64 * h:64 * h + 64, :].rearrange(
                                        "p (r n) -> p r n", r=2),
                                    in_=srcs[h][2 * w:2 * w + 2].rearrange(
                                        "r p n -> p r n"))
                        cst = consts[(p, w)]
                        soff, sidx = offsets[(a, p, w)]
                        n_slots, n_tok = cst["n_slots"], cst["n_tok"]
                        gi = gpool.tile([P, n_slots // 16], I16, tag="gi")
                        nc.sync.dma_start(
                            out=gi[:],
                            in_=gidx_rep[:, soff // 16:(soff + n_slots) // 16])
                        mi = gpool.tile([P, NPAD // 16], I16, tag="mi")
                        nc.sync.dma_start(
                            out=mi[:],
                            in_=midx_rep[:, sidx * NPAD // 16:
                                         (sidx + 1) * NPAD // 16])
                        rt = rpool.tile([P, n_tok], F32, tag="rt")
                        dump_rt = DEBUG and l == 0 and p == 0 and w == 0
                        for (s0, ni, t0, cb, K) in cst["chunks"]:
                            ch = gpool.tile([P, CH_SL], F32, tag="ch")
                            nc.gpsimd.ap_gather(
                                out_ap=ch[:, :ni].rearrange(
                                    "p (n d) -> p n d", d=1),
                                in_ap=tt[:].rearrange("p (n d) -> p n d", d=1),
                                idxs_ap=gi[:, s0 // 16:(s0 + ni) // 16],
                                channels=P, num_elems=WIN, d=1, num_idxs=ni)
                            vt = gpool.tile([P, CH_SL], F32, tag="vt")
                            for h in range(2):
                                nc.sync.dma_start(
                                    out=vt[64 * h:64 * h + 64, :ni],
                                    in_=t_gval[h, soff + s0:soff + s0 + ni]
                                    .unsqueeze(0).to_broadcast([64, ni]))
                            nc.vector.tensor_tensor(
                                out=ch[:, :ni], in0=ch[:, :ni], in1=vt[:, :ni],
                                op=mybir.AluOpType.mult)
                            nc.vector.tensor_reduce(
                                out=rt[:, t0:t0 + cb],
                                in_=ch[:, :ni].rearrange(
                                    "p (g k) -> p g k", k=K),
                                axis=mybir.AxisListType.X,
                                op=mybir.AluOpType.add)
                        if dump_rt:
                            nc.sync.dma_start(out=t_drt[:, :n_tok],
                                              in_=rt[:])
                        lhs = ident2 if l == 0 else wst[(l, p)]
                        for n0, nn in mm_chunks:
                            tmp = spool.tile([P, MCH], F32, tag="tmp")
                            nc.gpsimd.ap_gather(
                                out_ap=tmp[:, :nn].rearrange(
                                    "p (n d) -> p n d", d=1),
                                in_ap=rt[:].rearrange("p (n d) -> p n d", d=1),
                                idxs_ap=mi[:, n0 // 16:(n0 + nn) // 16],
                                channels=P, num_elems=n_tok, d=1, num_idxs=nn)
                            ps = psum.tile([D, MCH], F32, tag="ps")
                            nc.tensor.matmul(out=ps[:, :nn], lhsT=lhs[:],
                                             rhs=tmp[:, :nn],
                                             start=True, stop=True)
                            nc.vector.tensor_tensor(
                                out=hpre[p][:, n0:n0 + nn],
                                in0=hpre[p][:, n0:n0 + nn],
                                in1=ps[:, :nn], op=mybir.AluOpType.add)
                for i in range(2):
                    if l < 3:
                        nc.scalar.activation(
                            out=hpre[i][:], in_=hpre[i][:],
                            func=mybir.ActivationFunctionType.Relu)
                        nc.sync.dma_start(out=h_bounce[(l, i)][:],
                                          in_=hpre[i][:])
                        if DEBUG and l == 0:
                            nc.sync.dma_start(out=t_dh0[i], in_=hpre[i][:])
                        nc.gpsimd.collective_compute(
                            "AllGather", mybir.AluOpType.bypass,
                            replica_groups=[list(range(NC))],
                            ins=[h_bounce[(l, i)].opt()],
                            outs=[h_full[(l, i)].opt()])
                    else:
                        nc.sync.dma_start(out=t_out[i], in_=hpre[i][:])
    nc.compile()
    return nc


# --------------------------------------------------------------------------
# entry point
# --------------------------------------------------------------------------

def kernel(feat, adj1_rows, adj1_cols, adj1_vals,
           adj2_rows, adj2_cols, adj2_vals, W1, W2, W3, W4,
           _trace=False):
    global _CONSTS
    feat = np.asarray(feat, np.float32)

    s1 = _build_structure(adj1_rows, adj1_cols, adj1_vals)
    s2 = _build_structure(adj2_rows, adj2_cols, adj2_vals)
    offsets, s_tot, n_sub, gidx_t, gval_t, midx_t = _concat_structures([s1, s2])
    _CONSTS = ([s1[0], s2[0]], offsets, s_tot, n_sub)

    nc = build_program()

    in_maps = []
    for c in range(NC):
        ft = np.zeros((2, F_IN, NPAD), np.float32)
        ft[:, :, :NP] = feat[:, c * NP:(c + 1) * NP, :].transpose(0, 2, 1)
        in_maps.append({
            "feat": ft,
            "W1": np.asarray(W1, np.float32), "W2": np.asarray(W2, np.float32),
            "W3": np.asarray(W3, np.float32), "W4": np.asarray(W4, np.float32),
            "gidx": gidx_t[c], "gval": gval_t[c], "midx": midx_t[c],
        })
    res = run_bass_kernel_spmd(nc, in_maps, core_ids=list(range(NC)),
                               trace=_trace)
    kernel._nc = nc
    kernel._in_maps = in_maps
    kernel._results = res.results
    out = np.zeros((2, N, D), np.float32)
    for c in range(NC):
        out[:, c * NP:(c + 1) * NP, :] = \
            res.results[c]["out"][:, :, :NP].transpose(0, 2, 1)
    if _trace:
        kernel._last_results = res
    return out


# revision 3
# speedup vs baseline: 1.0207x; 1.0207x over previous
"""Multi-relational GCN (4 layers) on 8 TRN2 cores — ap_gather pipeline.

Layout is transposed throughout: features on partitions, nodes/edge-slots on
the free dimension. Per-edge source rows are fetched with gpsimd.ap_gather
(SBUF free-dim gather, no DMA descriptors) from an SBUF-resident window of
the source table. Relations are processed in PAIRS (2i, 2i+1): partitions
0-63 carry relation 2i, 64-127 carry 2i+1, so all 8 GPSIMD cores are busy
and one stacked-weight matmul [W_2i; W_2i+1]^T @ tmp sums both relations.

Per (pair, window) subset: ELL (degree-bucketed, K slots per node,
contiguous) gather -> multiply by edge vals (broadcast-DMAed to partitions)
-> tensor_reduce over K -> per-window token table -> merge ap_gather
(token -> node) -> matmul accumulate into h_pre. Layer 0 gathers from
pre-transformed XW1_r tables and merges with a stacked-identity matmul.

h tables are exchanged via AllGather of [64, NPAD] shards into
[8, 64, NPAD] DRAM tables; windows (12544 nodes = 2 ranks) are DMAed into
SBUF per (pair, window).
"""
import ml_dtypes
import numpy as np

import concourse.bacc as bacc
import concourse.mybir as mybir
import concourse.tile as tile
from concourse.bass_utils import run_bass_kernel_spmd
from concourse.masks import make_identity

N = 50000
NC = 8
NP = N // NC              # 6250 nodes per core per type
NPAD = 6272               # 49*128, padded per-core node count
NPALL = NC * NPAD         # 50176
NWIN = 4
WIN = NPALL // NWIN       # 12544 = 2 ranks per window
P = 128
D = 64
F_IN = 128
RELS = 4
BUCKETS = (1, 2, 3, 4, 5, 6, 8, 10, 12, 16, 20, 24, 32, 48, 64, 96, 128)
CH_SL = 4096              # gather chunk slot budget
MCH = 512                 # merge / matmul node chunk

F32 = mybir.dt.float32
BF16 = mybir.dt.bfloat16
I16 = mybir.dt.int16


# --------------------------------------------------------------------------
# host-side structure builder
# --------------------------------------------------------------------------

def _wrap16(a):
    n = a.shape[0]
    assert n % 16 == 0
    return a.reshape(n // 16, 16).T


def _build_structure(rows, cols, vals):
    """ELL structure for one adjacency, paired relations, 4 windows.

    Returns (consts, per_core):
      consts[(p, w)] = dict(groups, slot_base, tok_base, n_slots, n_tok,
                            zero_tok, chunks)  — identical across cores/halves
      per_core[c][(p, w)] = dict(gidx [2, n_slots] i16, gval [2, n_slots] f32,
                                 midx [2, NPAD] i16)
    """
    rows = np.asarray(rows).astype(np.int64)
    cols = np.asarray(cols).astype(np.int64)
    vals = np.asarray(vals).astype(np.float32)

    work = {}
    gc_all = {}
    for r in range(RELS):
        p, h = r // 2, r % 2
        rr, cc, vv = rows[r], cols[r], vals[r]
        core = rr // NP
        srow = (cc // NP) * NPAD + (cc % NP)
        win = srow // WIN
        for c in range(NC):
            mc = core == c
            for w in range(NWIN):
                m = mc & (win == w)
                dst = (rr[m] - c * NP).astype(np.int64)
                src = (srow[m] - w * WIN).astype(np.int32)
                val = vv[m]
                order = np.argsort(dst, kind="stable")
                dst, src, val = dst[order], src[order], val[order]
                counts = np.bincount(dst, minlength=NP)
                nodes = np.nonzero(counts)[0]
                degs = counts[nodes]
                kbi = np.searchsorted(np.asarray(BUCKETS), degs)
                gc = np.bincount(kbi, minlength=len(BUCKETS))
                work[(p, w, c, h)] = (dst, src, val, counts, nodes, kbi)
                gc_all.setdefault((p, w), []).append(gc)

    consts = {}
    from math import gcd
    for (p, w), gcs in gc_all.items():
        gmax = np.max(np.stack(gcs), axis=0)
        groups, slot_base, tok_base = [], [], []
        s_off = t_off = 0
        for bi, K in enumerate(BUCKETS):
            if gmax[bi] == 0:
                groups.append(None)
                slot_base.append(-1)
                tok_base.append(-1)
                continue
            # ap_gather ucode mishandles idx slices starting off a 16-byte
            # (128-idx) boundary: round each group's node count so both the
            # group's slot span and every chunk boundary are 128-slot aligned
            per = 128 // gcd(K, 128)
            gpad = int(-(-max(gmax[bi], 16) // per) * per)
            groups.append((K, gpad))
            slot_base.append(s_off)
            tok_base.append(t_off)
            s_off += gpad * K
            t_off += gpad
        # dummy all-zero group: guarantees a zero token for absent nodes
        zero_tok = t_off
        groups.append((1, 128))
        slot_base.append(s_off)
        tok_base.append(t_off)
        s_off += 128
        t_off += 128
        # chunk plan: (s0, ni, t0, cb, K); starts stay 128-slot aligned
        chunks = []
        for bi, grp in enumerate(groups):
            if grp is None:
                continue
            K, gpad = grp
            per = 128 // gcd(K, 128)
            cb_full = min(gpad, max(per, (CH_SL // K) // per * per))
            g0 = 0
            while g0 < gpad:
                cb = min(cb_full, gpad - g0)
                chunks.append((slot_base[bi] + g0 * K, cb * K,
                               tok_base[bi] + g0, cb, K))
                g0 += cb
        consts[(p, w)] = dict(groups=groups, slot_base=slot_base,
                              tok_base=tok_base, n_slots=s_off, n_tok=t_off,
                              zero_tok=zero_tok, chunks=chunks)

    per_core = [dict() for _ in range(NC)]
    for (p, w, c, h), (dst, src, val, counts, nodes, kbi) in work.items():
        cst = consts[(p, w)]
        if (p, w) not in per_core[c]:
            per_core[c][(p, w)] = dict(
                gidx=np.zeros((2, cst["n_slots"]), np.int16),
                gval=np.zeros((2, cst["n_slots"]), np.float32),
                midx=np.full((2, NPAD), cst["zero_tok"], np.int16))
        d = per_core[c][(p, w)]
        m_of_node = np.zeros(NP, np.int64)
        b_of_node = np.zeros(NP, np.int64)
        for bi in range(len(BUCKETS)):
            if cst["groups"][bi] is None:
                continue
            gn = nodes[kbi == bi]
            m_of_node[gn] = np.arange(len(gn))
            b_of_node[gn] = bi
            d["midx"][h, gn] = cst["tok_base"][bi] + np.arange(len(gn))
        starts = np.zeros(NP + 1, np.int64)
        np.cumsum(counts, out=starts[1:])
        k_e = np.arange(len(dst)) - starts[dst]
        b_e = b_of_node[dst]
        K_e = np.asarray(BUCKETS)[b_e]
        pos = (np.asarray(cst["slot_base"])[b_e]
               + m_of_node[dst] * K_e + k_e)
        d["gidx"][h, pos] = src
        d["gval"][h, pos] = val
    return consts, per_core


def _concat_structures(structs):
    """-> (offsets[(a,p,w)] = (slot_off, subset_idx), s_tot,
           gidx_t, gval_t, midx_t per core)."""
    offsets = {}
    s_off = 0
    si = 0
    for a, (consts, _) in enumerate(structs):
        for p in range(2):
            for w in range(NWIN):
                offsets[(a, p, w)] = (s_off, si)
                s_off += consts[(p, w)]["n_slots"]
                si += 1
    n_sub = si
    gidx_t, gval_t, midx_t = [], [], []
    for c in range(NC):
        gi = np.zeros((2, s_off), np.int16)
        gv = np.zeros((2, s_off), np.float32)
        mi = np.zeros((2, n_sub * NPAD), np.int16)
        for a, (consts, per_core) in enumerate(structs):
            for p in range(2):
                for w in range(NWIN):
                    so, sidx = offsets[(a, p, w)]
                    d = per_core[c][(p, w)]
                    ns = d["gidx"].shape[1]
                    gi[:, so:so + ns] = d["gidx"]
                    gv[:, so:so + ns] = d["gval"]
                    mi[:, sidx * NPAD:(sidx + 1) * NPAD] = d["midx"]
        gidx_t.append(np.concatenate([_wrap16(gi[0]), _wrap16(gi[1])], 0).copy())
        gval_t.append(gv)
        midx_t.append(np.concatenate([_wrap16(mi[0]), _wrap16(mi[1])], 0).copy())
    return offsets, s_off, n_sub, gidx_t, gval_t, midx_t


# --------------------------------------------------------------------------
# device program
# --------------------------------------------------------------------------

_CONSTS = None
DEBUG = False


def build_program():
    structs_consts, offsets, s_tot, n_sub = _CONSTS
    nc = bacc.Bacc("TRN2", target_bir_lowering=False, debug=False,
                   num_devices=NC)
    t_feat = nc.dram_tensor("feat", [2, F_IN, NPAD], F32, kind="ExternalInput")
    t_w = [nc.dram_tensor(f"W{l + 1}", [RELS, F_IN if l == 0 else D, D], F32,
                          kind="ExternalInput") for l in range(4)]
    t_gidx = nc.dram_tensor("gidx", [32, s_tot // 16], I16, kind="ExternalInput")
    t_gval = nc.dram_tensor("gval", [2, s_tot], BF16, kind="ExternalInput")
    t_midx = nc.dram_tensor("midx", [32, n_sub * NPAD // 16], I16,
                            kind="ExternalInput")
    t_out = nc.dram_tensor("out", [2, D, NPAD], F32, kind="ExternalOutput")
    if DEBUG:
        t_dxw = nc.dram_tensor("dxw", [RELS, D, NPAD], F32,
                               kind="ExternalOutput")
        t_dh0 = nc.dram_tensor("dh0", [2, D, NPAD], F32, kind="ExternalOutput")
        t_drt = nc.dram_tensor("drt", [P, 8192], F32, kind="ExternalOutput")

    with tile.TileContext(nc, num_cores=NC) as tc:
        with tc.tile_pool(name="cpool", bufs=1) as cpool, \
             tc.tile_pool(name="tpool", bufs=1) as tpool, \
             tc.tile_pool(name="hpool", bufs=1) as hpool, \
             tc.tile_pool(name="rpool", bufs=1) as rpool, \
             tc.tile_pool(name="gpool", bufs=2) as gpool, \
             tc.tile_pool(name="spool", bufs=2) as spool, \
             tc.tile_pool(name="psum", bufs=8, space="PSUM") as psum, \
             tc.tile_pool(name="dram", bufs=1, space="DRAM") as dram:

            # stacked identity [I64; I64] for layer-0 pair merges
            ident = cpool.tile([P, P], F32, tag="ident")
            make_identity(nc, ident[:])
            ident2 = cpool.tile([P, D], F32, tag="ident2")
            nc.scalar.copy(out=ident2[0:D, :], in_=ident[0:D, 0:D])
            nc.scalar.copy(out=ident2[D:P, :], in_=ident[D:P, D:P])

            # weights: W1 per relation [128, 64]; W2..4 stacked per pair
            w1t = []
            for r in range(RELS):
                wt = cpool.tile([F_IN, D], F32, tag=f"w1_{r}")
                nc.sync.dma_start(out=wt[:], in_=t_w[0][r])
                w1t.append(wt)
            wst = {}
            for l in range(1, 4):
                for p in range(2):
                    wt = cpool.tile([P, D], F32, tag=f"w{l}_{p}")
                    nc.sync.dma_start(out=wt[0:D, :], in_=t_w[l][2 * p])
                    nc.sync.dma_start(out=wt[D:P, :], in_=t_w[l][2 * p + 1])
                    wst[(l, p)] = wt

            # replicate wrapped idx tensors to 128 partitions in DRAM
            gidx_rep = dram.tile([P, s_tot // 16], I16, tag="gidx_rep")
            midx_rep = dram.tile([P, n_sub * NPAD // 16], I16, tag="midx_rep")
            for h in range(2):
                for g in range(4):
                    pr = 64 * h + 16 * g
                    nc.sync.dma_start(out=gidx_rep[pr:pr + 16, :],
                                      in_=t_gidx[16 * h:16 * h + 16, :])
                    nc.sync.dma_start(out=midx_rep[pr:pr + 16, :],
                                      in_=t_midx[16 * h:16 * h + 16, :])

            # DRAM tables
            xw1_full = [dram.tile([NC, D, NPAD], F32, tag=f"xw1f{r}",
                                  name=f"xw1f{r}", addr_space="Shared")
                        for r in range(RELS)]
            xw1_bounce = [dram.tile([D, NPAD], F32, tag=f"xw1b{r}",
                                    name=f"xw1b{r}") for r in range(RELS)]
            h_full = {}
            h_bounce = {}
            for l in range(3):
                for j in range(2):
                    h_full[(l, j)] = dram.tile([NC, D, NPAD], F32,
                                               tag=f"hf{l}_{j}",
                                               name=f"hf{l}_{j}",
                                               addr_space="Shared")
                    h_bounce[(l, j)] = dram.tile([D, NPAD], F32,
                                                 tag=f"hb{l}_{j}",
                                                 name=f"hb{l}_{j}")

            mm_chunks = [(c * MCH, min(MCH, NPAD - c * MCH))
                         for c in range(-(-NPAD // MCH))]

            # ---- stage A: XW1_r = (feat_j @ W1_r)^T, AllGather ----
            for j in range(2):
                for n0, nn in mm_chunks:
                    fch = spool.tile([F_IN, MCH], F32, tag="fch")
                    nc.sync.dma_start(out=fch[:, :nn],
                                      in_=t_feat[j, :, n0:n0 + nn])
                    for r in (j, j + 2):
                        ps = psum.tile([D, MCH], F32, tag="ps")
                        nc.tensor.matmul(out=ps[:, :nn], lhsT=w1t[r][:],
                                         rhs=fch[:, :nn], start=True, stop=True)
                        sb = spool.tile([D, MCH], F32, tag="sb")
                        nc.scalar.copy(out=sb[:, :nn], in_=ps[:, :nn])
                        nc.sync.dma_start(out=xw1_bounce[r][:, n0:n0 + nn],
                                          in_=sb[:, :nn])
            for r in range(RELS):
                nc.gpsimd.collective_compute(
                    "AllGather", mybir.AluOpType.bypass,
                    replica_groups=[list(range(NC))],
                    ins=[xw1_bounce[r].opt()],
                    outs=[xw1_full[r].opt()])
            if DEBUG:
                for r in range(RELS):
                    nc.sync.dma_start(out=t_dxw[r], in_=xw1_bounce[r][:])

            # ---- stage B: 4 GCN layers ----
            for l in range(4):
                a = 0 if l < 2 else 1
                consts = structs_consts[a]
                hpre = []
                for p in range(2):
                    ht = hpool.tile([D, NPAD], F32, tag=f"hpre{p}")
                    nc.vector.memset(ht[:], 0.0)
                    hpre.append(ht)
                for w in range(NWIN):
                    for p in range(2):
                        # table tile: halves = the pair's two source tables
                        if l == 0 or p == 0:
                            tt = tpool.tile([P, WIN], F32, tag="tt")
                            if l == 0:
                                srcs = (xw1_full[2 * p], xw1_full[2 * p + 1])
                            else:
                                srcs = (h_full[(l - 1, 0)], h_full[(l - 1, 1)])
                            for h in range(2):
                                nc.sync.dma_start(
                                    out=tt[64 * h:64 * h + 64, :].rearrange(
                                        "p (r n) -> p r n", r=2),
                                    in_=srcs[h][2 * w:2 * w + 2].rearrange(
                                        "r p n -> p r n"))
                        cst = consts[(p, w)]
                        soff, sidx = offsets[(a, p, w)]
                        n_slots, n_tok = cst["n_slots"], cst["n_tok"]
                        gi = gpool.tile([P, n_slots // 16], I16, tag="gi")
                        nc.sync.dma_start(
                            out=gi[:],
                            in_=gidx_rep[:, soff // 16:(soff + n_slots) // 16])
                        mi = gpool.tile([P, NPAD // 16], I16, tag="mi")
                        nc.sync.dma_start(
                            out=mi[:],
                            in_=midx_rep[:, sidx * NPAD // 16:
                                         (sidx + 1) * NPAD // 16])
                        rt = rpool.tile([P, n_tok], F32, tag="rt")
                        dump_rt = DEBUG and l == 0 and p == 0 and w == 0
                        for (s0, ni, t0, cb, K) in cst["chunks"]:
                            ch = gpool.tile([P, CH_SL], F32, tag="ch")
                            nc.gpsimd.ap_gather(
                                out_ap=ch[:, :ni].rearrange(
                                    "p (n d) -> p n d", d=1),
                                in_ap=tt[:].rearrange("p (n d) -> p n d", d=1),
                                idxs_ap=gi[:, s0 // 16:(s0 + ni) // 16],
                                channels=P, num_elems=WIN, d=1, num_idxs=ni)
                            vt = gpool.tile([P, CH_SL], BF16, tag="vt")
                            for h in range(2):
                                nc.sync.dma_start(
                                    out=vt[64 * h:64 * h + 64, :ni],
                                    in_=t_gval[h, soff + s0:soff + s0 + ni]
                                    .unsqueeze(0).to_broadcast([64, ni]))
                            nc.vector.tensor_tensor(
                                out=ch[:, :ni], in0=ch[:, :ni], in1=vt[:, :ni],
                                op=mybir.AluOpType.mult)
                            nc.vector.tensor_reduce(
                                out=rt[:, t0:t0 + cb],
                                in_=ch[:, :ni].rearrange(
                                    "p (g k) -> p g k", k=K),
                                axis=mybir.AxisListType.X,
                                op=mybir.AluOpType.add)
                        if dump_rt:
                            nc.sync.dma_start(out=t_drt[:, :n_tok],
                                              in_=rt[:])
                        lhs = ident2 if l == 0 else wst[(l, p)]
                        for n0, nn in mm_chunks:
                            tmp = spool.tile([P, MCH], F32, tag="tmp")
                            nc.gpsimd.ap_gather(
                                out_ap=tmp[:, :nn].rearrange(
                                    "p (n d) -> p n d", d=1),
                                in_ap=rt[:].rearrange("p (n d) -> p n d", d=1),
                                idxs_ap=mi[:, n0 // 16:(n0 + nn) // 16],
                                channels=P, num_elems=n_tok, d=1, num_idxs=nn)
                            ps = psum.tile([D, MCH], F32, tag="ps")
                            nc.tensor.matmul(out=ps[:, :nn], lhsT=lhs[:],
                                             rhs=tmp[:, :nn],
                                             start=True, stop=True)
                            nc.vector.tensor_tensor(
                                out=hpre[p][:, n0:n0 + nn],
                                in0=hpre[p][:, n0:n0 + nn],
                                in1=ps[:, :nn], op=mybir.AluOpType.add)
                for i in range(2):
                    if l < 3:
                        nc.scalar.activation(
                            out=hpre[i][:], in_=hpre[i][:],
                            func=mybir.ActivationFunctionType.Relu)
                        nc.sync.dma_start(out=h_bounce[(l, i)][:],
                                          in_=hpre[i][:])
                        if DEBUG and l == 0:
                            nc.sync.dma_start(out=t_dh0[i], in_=hpre[i][:])
                        nc.gpsimd.collective_compute(
                            "AllGather", mybir.AluOpType.bypass,
                            replica_groups=[list(range(NC))],
                            ins=[h_bounce[(l, i)].opt()],
                            outs=[h_full[(l, i)].opt()])
                    else:
                        nc.sync.dma_start(out=t_out[i], in_=hpre[i][:])
    nc.compile()
    return nc


# --------------------------------------------------------------------------
# entry point
# --------------------------------------------------------------------------

def kernel(feat, adj1_rows, adj1_cols, adj1_vals,
           adj2_rows, adj2_cols, adj2_vals, W1, W2, W3, W4,
           _trace=False):
    global _CONSTS
    feat = np.asarray(feat, np.float32)

    s1 = _build_structure(adj1_rows, adj1_cols, adj1_vals)
    s2 = _build_structure(adj2_rows, adj2_cols, adj2_vals)
    offsets, s_tot, n_sub, gidx_t, gval_t, midx_t = _concat_structures([s1, s2])
    _CONSTS = ([s1[0], s2[0]], offsets, s_tot, n_sub)

    nc = build_program()

    in_maps = []
    for c in range(NC):
        ft = np.zeros((2, F_IN, NPAD), np.float32)
        ft[:, :, :NP] = feat[:, c * NP:(c + 1) * NP, :].transpose(0, 2, 1)
        in_maps.append({
            "feat": ft,
            "W1": np.asarray(W1, np.float32), "W2": np.asarray(W2, np.float32),
            "W3": np.asarray(W3, np.float32), "W4": np.asarray(W4, np.float32),
            "gidx": gidx_t[c],
            "gval": gval_t[c].astype(ml_dtypes.bfloat16),
            "midx": midx_t[c],
        })
    res = run_bass_kernel_spmd(nc, in_maps, core_ids=list(range(NC)),
                               trace=_trace)
    kernel._nc = nc
    kernel._in_maps = in_maps
    kernel._results = res.results
    out = np.zeros((2, N, D), np.float32)
    for c in range(NC):
        out[:, c * NP:(c + 1) * NP, :] = \
            res.results[c]["out"][:, :, :NP].transpose(0, 2, 1)
    if _trace:
        kernel._last_results = res
    return out


# revision 4
# speedup vs baseline: 1.0380x; 1.0169x over previous
"""Multi-relational GCN (4 layers) on 8 TRN2 cores — ap_gather pipeline.

Layout is transposed throughout: features on partitions, nodes/edge-slots on
the free dimension. Per-edge source rows are fetched with gpsimd.ap_gather
(SBUF free-dim gather, no DMA descriptors) from an SBUF-resident window of
the source table. Relations are processed in PAIRS (2i, 2i+1): partitions
0-63 carry relation 2i, 64-127 carry 2i+1, so all 8 GPSIMD cores are busy
and one stacked-weight matmul [W_2i; W_2i+1]^T @ tmp sums both relations.

Per (pair, window) subset: ELL (degree-bucketed, K slots per node,
contiguous) gather -> multiply by edge vals (broadcast-DMAed to partitions)
-> tensor_reduce over K -> per-window token table -> merge ap_gather
(token -> node) -> matmul accumulate into h_pre. Layer 0 gathers from
pre-transformed XW1_r tables and merges with a stacked-identity matmul.

h tables are exchanged via AllGather of [64, NPAD] shards into
[8, 64, NPAD] DRAM tables; windows (12544 nodes = 2 ranks) are DMAed into
SBUF per (pair, window).
"""
import ml_dtypes
import numpy as np

import concourse.bacc as bacc
import concourse.mybir as mybir
import concourse.tile as tile
from concourse.bass_utils import run_bass_kernel_spmd
from concourse.masks import make_identity

N = 50000
NC = 8
NP = N // NC              # 6250 nodes per core per type
NPAD = 6272               # 49*128, padded per-core node count
NPALL = NC * NPAD         # 50176
NWIN = 4
WIN = NPALL // NWIN       # 12544 = 2 ranks per window
P = 128
D = 64
F_IN = 128
RELS = 4
BUCKETS = (1, 2, 3, 4, 5, 6, 8, 10, 12, 16, 20, 24, 32, 48, 64, 96, 128)
CH_SL = 3584              # gather chunk slot budget
MCH = 512                 # merge / matmul node chunk

F32 = mybir.dt.float32
BF16 = mybir.dt.bfloat16
I16 = mybir.dt.int16


# --------------------------------------------------------------------------
# host-side structure builder
# --------------------------------------------------------------------------

def _wrap16(a):
    n = a.shape[0]
    assert n % 16 == 0
    return a.reshape(n // 16, 16).T


def _build_structure(rows, cols, vals):
    """ELL structure for one adjacency, paired relations, 4 windows.

    Returns (consts, per_core):
      consts[(p, w)] = dict(groups, slot_base, tok_base, n_slots, n_tok,
                            zero_tok, chunks)  — identical across cores/halves
      per_core[c][(p, w)] = dict(gidx [2, n_slots] i16, gval [2, n_slots] f32,
                                 midx [2, NPAD] i16)
    """
    rows = np.asarray(rows).astype(np.int64)
    cols = np.asarray(cols).astype(np.int64)
    vals = np.asarray(vals).astype(np.float32)

    work = {}
    gc_all = {}
    for r in range(RELS):
        p, h = r // 2, r % 2
        rr, cc, vv = rows[r], cols[r], vals[r]
        core = rr // NP
        srow = (cc // NP) * NPAD + (cc % NP)
        win = srow // WIN
        for c in range(NC):
            mc = core == c
            for w in range(NWIN):
                m = mc & (win == w)
                dst = (rr[m] - c * NP).astype(np.int64)
                src = (srow[m] - w * WIN).astype(np.int32)
                val = vv[m]
                order = np.argsort(dst, kind="stable")
                dst, src, val = dst[order], src[order], val[order]
                counts = np.bincount(dst, minlength=NP)
                nodes = np.nonzero(counts)[0]
                degs = counts[nodes]
                kbi = np.searchsorted(np.asarray(BUCKETS), degs)
                gc = np.bincount(kbi, minlength=len(BUCKETS))
                work[(p, w, c, h)] = (dst, src, val, counts, nodes, kbi)
                gc_all.setdefault((p, w), []).append(gc)

    consts = {}
    from math import gcd
    for (p, w), gcs in gc_all.items():
        gmax = np.max(np.stack(gcs), axis=0)
        groups, slot_base, tok_base = [], [], []
        s_off = t_off = 0
        for bi, K in enumerate(BUCKETS):
            if gmax[bi] == 0:
                groups.append(None)
                slot_base.append(-1)
                tok_base.append(-1)
                continue
            # ap_gather ucode mishandles idx slices starting off a 16-byte
            # (128-idx) boundary: round each group's node count so both the
            # group's slot span and every chunk boundary are 128-slot aligned
            per = 128 // gcd(K, 128)
            gpad = int(-(-max(gmax[bi], 16) // per) * per)
            groups.append((K, gpad))
            slot_base.append(s_off)
            tok_base.append(t_off)
            s_off += gpad * K
            t_off += gpad
        # dummy all-zero group: guarantees a zero token for absent nodes
        zero_tok = t_off
        groups.append((1, 128))
        slot_base.append(s_off)
        tok_base.append(t_off)
        s_off += 128
        t_off += 128
        # chunk plan: (s0, ni, t0, cb, K); starts stay 128-slot aligned
        chunks = []
        for bi, grp in enumerate(groups):
            if grp is None:
                continue
            K, gpad = grp
            per = 128 // gcd(K, 128)
            cb_full = min(gpad, max(per, (CH_SL // K) // per * per))
            g0 = 0
            while g0 < gpad:
                cb = min(cb_full, gpad - g0)
                chunks.append((slot_base[bi] + g0 * K, cb * K,
                               tok_base[bi] + g0, cb, K))
                g0 += cb
        consts[(p, w)] = dict(groups=groups, slot_base=slot_base,
                              tok_base=tok_base, n_slots=s_off, n_tok=t_off,
                              zero_tok=zero_tok, chunks=chunks)

    per_core = [dict() for _ in range(NC)]
    for (p, w, c, h), (dst, src, val, counts, nodes, kbi) in work.items():
        cst = consts[(p, w)]
        if (p, w) not in per_core[c]:
            per_core[c][(p, w)] = dict(
                gidx=np.zeros((2, cst["n_slots"]), np.int16),
                gval=np.zeros((2, cst["n_slots"]), np.float32),
                midx=np.full((2, NPAD), cst["zero_tok"], np.int16))
        d = per_core[c][(p, w)]
        m_of_node = np.zeros(NP, np.int64)
        b_of_node = np.zeros(NP, np.int64)
        for bi in range(len(BUCKETS)):
            if cst["groups"][bi] is None:
                continue
            gn = nodes[kbi == bi]
            m_of_node[gn] = np.arange(len(gn))
            b_of_node[gn] = bi
            d["midx"][h, gn] = cst["tok_base"][bi] + np.arange(len(gn))
        starts = np.zeros(NP + 1, np.int64)
        np.cumsum(counts, out=starts[1:])
        k_e = np.arange(len(dst)) - starts[dst]
        b_e = b_of_node[dst]
        K_e = np.asarray(BUCKETS)[b_e]
        pos = (np.asarray(cst["slot_base"])[b_e]
               + m_of_node[dst] * K_e + k_e)
        d["gidx"][h, pos] = src
        d["gval"][h, pos] = val
    return consts, per_core


def _concat_structures(structs):
    """-> (offsets[(a,p,w)] = (slot_off, subset_idx), s_tot,
           gidx_t, gval_t, midx_t per core)."""
    offsets = {}
    s_off = 0
    si = 0
    for a, (consts, _) in enumerate(structs):
        for p in range(2):
            for w in range(NWIN):
                offsets[(a, p, w)] = (s_off, si)
                s_off += consts[(p, w)]["n_slots"]
                si += 1
    n_sub = si
    gidx_t, gval_t, midx_t = [], [], []
    for c in range(NC):
        gi = np.zeros((2, s_off), np.int16)
        gv = np.zeros((2, s_off), np.float32)
        mi = np.zeros((2, n_sub * NPAD), np.int16)
        for a, (consts, per_core) in enumerate(structs):
            for p in range(2):
                for w in range(NWIN):
                    so, sidx = offsets[(a, p, w)]
                    d = per_core[c][(p, w)]
                    ns = d["gidx"].shape[1]
                    gi[:, so:so + ns] = d["gidx"]
                    gv[:, so:so + ns] = d["gval"]
                    mi[:, sidx * NPAD:(sidx + 1) * NPAD] = d["midx"]
        gidx_t.append(np.concatenate([_wrap16(gi[0]), _wrap16(gi[1])], 0).copy())
        gval_t.append(gv)
        midx_t.append(np.concatenate([_wrap16(mi[0]), _wrap16(mi[1])], 0).copy())
    return offsets, s_off, n_sub, gidx_t, gval_t, midx_t


# --------------------------------------------------------------------------
# device program
# --------------------------------------------------------------------------

_CONSTS = None
DEBUG = False


def build_program():
    structs_consts, offsets, s_tot, n_sub = _CONSTS
    nc = bacc.Bacc("TRN2", target_bir_lowering=False, debug=False,
                   num_devices=NC)
    t_feat = nc.dram_tensor("feat", [2, F_IN, NPAD], F32, kind="ExternalInput")
    t_w = [nc.dram_tensor(f"W{l + 1}", [RELS, F_IN if l == 0 else D, D], F32,
                          kind="ExternalInput") for l in range(4)]
    t_gidx = nc.dram_tensor("gidx", [32, s_tot // 16], I16, kind="ExternalInput")
    t_gval = nc.dram_tensor("gval", [2, s_tot], BF16, kind="ExternalInput")
    t_midx = nc.dram_tensor("midx", [32, n_sub * NPAD // 16], I16,
                            kind="ExternalInput")
    t_out = nc.dram_tensor("out", [2, D, NPAD], F32, kind="ExternalOutput")
    if DEBUG:
        t_dxw = nc.dram_tensor("dxw", [RELS, D, NPAD], F32,
                               kind="ExternalOutput")
        t_dh0 = nc.dram_tensor("dh0", [2, D, NPAD], F32, kind="ExternalOutput")
        t_drt = nc.dram_tensor("drt", [P, 8192], F32, kind="ExternalOutput")

    with tile.TileContext(nc, num_cores=NC) as tc:
        with tc.tile_pool(name="cpool", bufs=1) as cpool, \
             tc.tile_pool(name="tpool", bufs=1) as tpool, \
             tc.tile_pool(name="hpool", bufs=1) as hpool, \
             tc.tile_pool(name="rpool", bufs=1) as rpool, \
             tc.tile_pool(name="gpool", bufs=2) as gpool, \
             tc.tile_pool(name="spool", bufs=2) as spool, \
             tc.tile_pool(name="psum", bufs=8, space="PSUM") as psum, \
             tc.tile_pool(name="dram", bufs=1, space="DRAM") as dram:

            # stacked identity [I64; I64] for layer-0 pair merges
            ident = cpool.tile([P, P], F32, tag="ident")
            make_identity(nc, ident[:])
            ident2 = cpool.tile([P, D], F32, tag="ident2")
            nc.scalar.copy(out=ident2[0:D, :], in_=ident[0:D, 0:D])
            nc.scalar.copy(out=ident2[D:P, :], in_=ident[D:P, D:P])

            # weights: W1 per relation [128, 64]; W2..4 stacked per pair
            w1t = []
            for r in range(RELS):
                wt = cpool.tile([F_IN, D], F32, tag=f"w1_{r}")
                nc.sync.dma_start(out=wt[:], in_=t_w[0][r])
                w1t.append(wt)
            wst = {}
            for l in range(1, 4):
                for p in range(2):
                    wt = cpool.tile([P, D], F32, tag=f"w{l}_{p}")
                    nc.sync.dma_start(out=wt[0:D, :], in_=t_w[l][2 * p])
                    nc.sync.dma_start(out=wt[D:P, :], in_=t_w[l][2 * p + 1])
                    wst[(l, p)] = wt

            # replicate wrapped idx tensors to 128 partitions in DRAM
            gidx_rep = dram.tile([P, s_tot // 16], I16, tag="gidx_rep")
            midx_rep = dram.tile([P, n_sub * NPAD // 16], I16, tag="midx_rep")
            for h in range(2):
                for g in range(4):
                    pr = 64 * h + 16 * g
                    nc.sync.dma_start(out=gidx_rep[pr:pr + 16, :],
                                      in_=t_gidx[16 * h:16 * h + 16, :])
                    nc.sync.dma_start(out=midx_rep[pr:pr + 16, :],
                                      in_=t_midx[16 * h:16 * h + 16, :])

            # DRAM tables
            xw1_full = [dram.tile([NC, D, NPAD], F32, tag=f"xw1f{r}",
                                  name=f"xw1f{r}", addr_space="Shared")
                        for r in range(RELS)]
            xw1_bounce = [dram.tile([D, NPAD], F32, tag=f"xw1b{r}",
                                    name=f"xw1b{r}") for r in range(RELS)]
            h_full = {}
            h_bounce = {}
            for l in range(3):
                for j in range(2):
                    h_full[(l, j)] = dram.tile([NC, D, NPAD], F32,
                                               tag=f"hf{l}_{j}",
                                               name=f"hf{l}_{j}",
                                               addr_space="Shared")
                    h_bounce[(l, j)] = dram.tile([D, NPAD], F32,
                                                 tag=f"hb{l}_{j}",
                                                 name=f"hb{l}_{j}")

            mm_chunks = [(c * MCH, min(MCH, NPAD - c * MCH))
                         for c in range(-(-NPAD // MCH))]

            # ---- stage A: XW1_r = (feat_j @ W1_r)^T, AllGather ----
            for j in range(2):
                for n0, nn in mm_chunks:
                    fch = spool.tile([F_IN, MCH], F32, tag="fch")
                    nc.sync.dma_start(out=fch[:, :nn],
                                      in_=t_feat[j, :, n0:n0 + nn])
                    for r in (j, j + 2):
                        ps = psum.tile([D, MCH], F32, tag="ps")
                        nc.tensor.matmul(out=ps[:, :nn], lhsT=w1t[r][:],
                                         rhs=fch[:, :nn], start=True, stop=True)
                        sb = spool.tile([D, MCH], F32, tag="sb")
                        nc.scalar.copy(out=sb[:, :nn], in_=ps[:, :nn])
                        nc.sync.dma_start(out=xw1_bounce[r][:, n0:n0 + nn],
                                          in_=sb[:, :nn])
            for r in range(RELS):
                nc.gpsimd.collective_compute(
                    "AllGather", mybir.AluOpType.bypass,
                    replica_groups=[list(range(NC))],
                    ins=[xw1_bounce[r].opt()],
                    outs=[xw1_full[r].opt()])
            if DEBUG:
                for r in range(RELS):
                    nc.sync.dma_start(out=t_dxw[r], in_=xw1_bounce[r][:])

            # ---- stage B: 4 GCN layers ----
            for l in range(4):
                a = 0 if l < 2 else 1
                consts = structs_consts[a]
                hpre = []
                for p in range(2):
                    ht = hpool.tile([D, NPAD], F32, tag=f"hpre{p}")
                    nc.vector.memset(ht[:], 0.0)
                    hpre.append(ht)
                for w in range(NWIN):
                    for p in range(2):
                        # table tile: halves = the pair's two source tables
                        if l == 0 or p == 0:
                            tt = tpool.tile([P, WIN], F32, tag="tt")
                            if l == 0:
                                srcs = (xw1_full[2 * p], xw1_full[2 * p + 1])
                            else:
                                srcs = (h_full[(l - 1, 0)], h_full[(l - 1, 1)])
                            for h in range(2):
                                nc.sync.dma_start(
                                    out=tt[64 * h:64 * h + 64, :].rearrange(
                                        "p (r n) -> p r n", r=2),
                                    in_=srcs[h][2 * w:2 * w + 2].rearrange(
                                        "r p n -> p r n"))
                        cst = consts[(p, w)]
                        soff, sidx = offsets[(a, p, w)]
                        n_slots, n_tok = cst["n_slots"], cst["n_tok"]
                        gi = gpool.tile([P, n_slots // 16], I16, tag="gi")
                        nc.sync.dma_start(
                            out=gi[:],
                            in_=gidx_rep[:, soff // 16:(soff + n_slots) // 16])
                        mi = gpool.tile([P, NPAD // 16], I16, tag="mi")
                        nc.sync.dma_start(
                            out=mi[:],
                            in_=midx_rep[:, sidx * NPAD // 16:
                                         (sidx + 1) * NPAD // 16])
                        rt = rpool.tile([P, n_tok], F32, tag="rt")
                        dump_rt = DEBUG and l == 0 and p == 0 and w == 0
                        for (s0, ni, t0, cb, K) in cst["chunks"]:
                            ch = gpool.tile([P, CH_SL], F32, tag="ch")
                            nc.gpsimd.ap_gather(
                                out_ap=ch[:, :ni].rearrange(
                                    "p (n d) -> p n d", d=1),
                                in_ap=tt[:].rearrange("p (n d) -> p n d", d=1),
                                idxs_ap=gi[:, s0 // 16:(s0 + ni) // 16],
                                channels=P, num_elems=WIN, d=1, num_idxs=ni)
                            vt = gpool.tile([P, CH_SL], BF16, tag="vt")
                            for h in range(2):
                                nc.sync.dma_start(
                                    out=vt[64 * h:64 * h + 64, :ni],
                                    in_=t_gval[h, soff + s0:soff + s0 + ni]
                                    .unsqueeze(0).to_broadcast([64, ni]))
                            nc.vector.tensor_tensor(
                                out=ch[:, :ni], in0=ch[:, :ni], in1=vt[:, :ni],
                                op=mybir.AluOpType.mult)
                            nc.vector.tensor_reduce(
                                out=rt[:, t0:t0 + cb],
                                in_=ch[:, :ni].rearrange(
                                    "p (g k) -> p g k", k=K),
                                axis=mybir.AxisListType.X,
                                op=mybir.AluOpType.add)
                        if dump_rt:
                            nc.sync.dma_start(out=t_drt[:, :n_tok],
                                              in_=rt[:])
                        lhs = ident2 if l == 0 else wst[(l, p)]
                        mt = rpool.tile([P, NPAD], F32, tag="mt")
                        nc.gpsimd.ap_gather(
                            out_ap=mt[:].rearrange("p (n d) -> p n d", d=1),
                            in_ap=rt[:].rearrange("p (n d) -> p n d", d=1),
                            idxs_ap=mi[:],
                            channels=P, num_elems=n_tok, d=1, num_idxs=NPAD)
                        for n0, nn in mm_chunks:
                            ps = psum.tile([D, MCH], F32, tag="ps")
                            nc.tensor.matmul(out=ps[:, :nn], lhsT=lhs[:],
                                             rhs=mt[:, n0:n0 + nn],
                                             start=True, stop=True)
                            nc.vector.tensor_tensor(
                                out=hpre[p][:, n0:n0 + nn],
                                in0=hpre[p][:, n0:n0 + nn],
                                in1=ps[:, :nn], op=mybir.AluOpType.add)
                for i in range(2):
                    if l < 3:
                        nc.scalar.activation(
                            out=hpre[i][:], in_=hpre[i][:],
                            func=mybir.ActivationFunctionType.Relu)
                        nc.sync.dma_start(out=h_bounce[(l, i)][:],
                                          in_=hpre[i][:])
                        if DEBUG and l == 0:
                            nc.sync.dma_start(out=t_dh0[i], in_=hpre[i][:])
                        nc.gpsimd.collective_compute(
                            "AllGather", mybir.AluOpType.bypass,
                            replica_groups=[list(range(NC))],
                            ins=[h_bounce[(l, i)].opt()],
                            outs=[h_full[(l, i)].opt()])
                    else:
                        nc.sync.dma_start(out=t_out[i], in_=hpre[i][:])
    nc.compile()
    return nc


# --------------------------------------------------------------------------
# entry point
# --------------------------------------------------------------------------

def kernel(feat, adj1_rows, adj1_cols, adj1_vals,
           adj2_rows, adj2_cols, adj2_vals, W1, W2, W3, W4,
           _trace=False):
    global _CONSTS
    feat = np.asarray(feat, np.float32)

    s1 = _build_structure(adj1_rows, adj1_cols, adj1_vals)
    s2 = _build_structure(adj2_rows, adj2_cols, adj2_vals)
    offsets, s_tot, n_sub, gidx_t, gval_t, midx_t = _concat_structures([s1, s2])
    _CONSTS = ([s1[0], s2[0]], offsets, s_tot, n_sub)

    nc = build_program()

    in_maps = []
    for c in range(NC):
        ft = np.zeros((2, F_IN, NPAD), np.float32)
        ft[:, :, :NP] = feat[:, c * NP:(c + 1) * NP, :].transpose(0, 2, 1)
        in_maps.append({
            "feat": ft,
            "W1": np.asarray(W1, np.float32), "W2": np.asarray(W2, np.float32),
            "W3": np.asarray(W3, np.float32), "W4": np.asarray(W4, np.float32),
            "gidx": gidx_t[c],
            "gval": gval_t[c].astype(ml_dtypes.bfloat16),
            "midx": midx_t[c],
        })
    res = run_bass_kernel_spmd(nc, in_maps, core_ids=list(range(NC)),
                               trace=_trace)
    kernel._nc = nc
    kernel._in_maps = in_maps
    kernel._results = res.results
    out = np.zeros((2, N, D), np.float32)
    for c in range(NC):
        out[:, c * NP:(c + 1) * NP, :] = \
            res.results[c]["out"][:, :, :NP].transpose(0, 2, 1)
    if _trace:
        kernel._last_results = res
    return out
